# revision 1
# baseline (speedup 1.0000x reference)
"""Causal self-attention for B=4, L=2048, D=768, H=6 on 8 TRN2 NeuronCores.

Sharding: 8 cores = 4 batches x 2 head-groups (3 heads / 384 hidden each).
Each core computes, for its (batch, head-group):
  QT/KT = (x @ W{q,k})^T + b            [128d x L per head, fp32r]
  V     = x @ Wv                        [L x 384, fp32r]
  per head, per 512-wide q-group:
    S^T  = K_blk @ Q^T                  (PE, fp32r, causal block-skip)
    A^T  = exp(S^T / sqrt(128))         (ACT, masked on diagonal blocks)
    O^T += V_blk^T @ A^T                (PE)  + row-sums via ones-matmul
    O^T *= 1/sums  (sums broadcast over partitions via rank-1 matmul)
  Y_part = O @ Wo_slice                 [L x 768 partial]
Host sums the two head-group partials per batch and adds the bias terms
(bv @ Wo + bo); bq/bk are applied on-device (zero-cost per-partition add).

All matmuls run in float32r (full PE rate, ~1e-4 rel err); softmax math in
fp32. exp needs no max-subtraction: scores/sqrt(128) stay in [-10, 10] for
normally-distributed inputs, well inside fp32 exp range.
"""

import math

import numpy as np

import concourse.bacc as bacc
import concourse.mybir as mybir
import concourse.tile as tile
from concourse.bass_utils import run_bass_kernel_spmd

F32 = mybir.dt.float32
F32R = mybir.dt.float32r
EXP = mybir.ActivationFunctionType.Exp

B = 4
L = 2048
D = 768
HEADS = 6
HD = 128
HPC = 3          # heads per core
HG = HPC * HD    # 384: per-core slice of the hidden dim
CB = D // 128    # 6 contraction chunks
SCALE = 1.0 / math.sqrt(HD)
N_CORES = 8


def build_nc(L_=L):
    """Build + compile the per-core Bass program (same program on all cores)."""
    LBn = L_ // 128   # 128-row L blocks
    NQG = L_ // 512   # 512-wide q groups

    nc = bacc.Bacc("TRN2", target_bir_lowering=False, debug=False)
    x_d = nc.dram_tensor("x", [L_, D], F32, kind="ExternalInput").ap()
    wq_d = nc.dram_tensor("wq", [D, HG], F32, kind="ExternalInput").ap()
    wk_d = nc.dram_tensor("wk", [D, HG], F32, kind="ExternalInput").ap()
    wv_d = nc.dram_tensor("wv", [D, HG], F32, kind="ExternalInput").ap()
    wo_d = nc.dram_tensor("wo", [HG, D], F32, kind="ExternalInput").ap()
    bq_d = nc.dram_tensor("bq", [HG], F32, kind="ExternalInput").ap()
    bk_d = nc.dram_tensor("bk", [HG], F32, kind="ExternalInput").ap()
    ident_d = nc.dram_tensor("ident", [128, 128], F32, kind="ExternalInput").ap()
    maskf_d = nc.dram_tensor("maskf", [128, 896], F32, kind="ExternalInput").ap()
    y_d = nc.dram_tensor("y", [L_, D], F32, kind="ExternalOutput").ap()

    with tile.TileContext(nc) as tc:
        with (
            tc.tile_pool(name="persist", bufs=1) as pp,
            tc.tile_pool(name="qkv_sb", bufs=1) as pqkv,
        ):
            # constants go on the SWDGE (gpsimd) queue so the HWDGE queue's
            # first descriptors are the x chunks the PE transposes wait on
            ident = pp.tile([128, 128], F32R)
            nc.gpsimd.dma_start(ident, ident_d.bitcast(F32R))
            maskf = pp.tile([128, 896], F32R)
            bq_sb = pp.tile([128, HPC], F32)
            bk_sb = pp.tile([128, HPC], F32)
            nc.gpsimd.dma_start(bq_sb, bq_d.rearrange("(h p) -> p h", p=128))
            nc.gpsimd.dma_start(bk_sb, bk_d.rearrange("(h p) -> p h", p=128))
            # dummy exp: pulls the ACT Exp-table load off the QKV->attention
            # transition and into the startup DMA shadow
            warm = pp.tile([1, 1], F32)
            nc.scalar.activation(warm, ident[:1, :1], EXP)

            q_t = pqkv.tile([128, HPC, L_], F32R)   # Q^T: [d, (head, L)]
            k_t = pqkv.tile([128, HPC, L_], F32R)   # K^T
            v_t = pqkv.tile([128, LBn, HG], F32R)   # V:  [k-in-block, (block, hd)]
            o_t = pqkv.tile([128, HPC, L_], F32R)   # O^T (normalized)

            # ---- phase 1: load x, transpose to x^T, QKV projections ----
            with (
                tc.tile_pool(name="w_sb", bufs=1) as pw,
                tc.tile_pool(name="x_nat", bufs=8) as px,
                tc.tile_pool(name="xT", bufs=2) as pxt,
                tc.tile_pool(name="ps_t", bufs=2, space="PSUM") as ps_t,
                tc.tile_pool(name="ps_qk", bufs=2, space="PSUM") as ps_qk,
                tc.tile_pool(name="ps_v", bufs=2, space="PSUM") as ps_v,
            ):
                # per-128-row x tiles: fine-grained DMA→transpose pipelining
                def load_xb(g, b):
                    # alternate between the two HWDGE queues (SP / Activation)
                    # to parallelize descriptor generation and transfers
                    xb = px.tile([128, D], F32R, tag="xn")
                    r0 = g * 512 + b * 128
                    eng = nc.scalar if (g == 0 and b % 2 == 1) else nc.sync
                    eng.dma_start(
                        xb,
                        x_d.bitcast(F32R)[r0 : r0 + 128, :].rearrange(
                            "(o p) c -> p o c", p=128
                        )[:, 0],
                    )
                    return xb

                xbs = [load_xb(0, b) for b in range(4)]
                wq_sb = pw.tile([128, CB, HG], F32R)
                wk_sb = pw.tile([128, CB, HG], F32R)
                wv_sb = pw.tile([128, CB, HG], F32R)
                for w_sb, w_d in ((wq_sb, wq_d), (wk_sb, wk_d), (wv_sb, wv_d)):
                    nc.sync.dma_start(
                        w_sb, w_d.bitcast(F32R).rearrange("(c p) d -> p c d", p=128)
                    )

                def emit_transposes(g, xn):
                    # b-major groups: each PSUM group + copy depends on a
                    # single x row-block DMA, so the PE transposes stream in
                    # lockstep with the arriving sub-DMAs
                    xt = pxt.tile([128, CB, 512], F32R, name="xt")
                    for b in range(4):
                        for c0, cw in ((0, 4), (4, 2)):
                            pt = ps_t.tile([128, cw, 128], F32R, name="pt")
                            for ci in range(cw):
                                c = c0 + ci
                                nc.tensor.transpose(
                                    pt[:, ci, :],
                                    xn[b][:, c * 128 : (c + 1) * 128],
                                    ident,
                                )
                            nc.vector.tensor_copy(
                                xt[:, c0 : c0 + cw, b * 128 : (b + 1) * 128], pt
                            )
                    return xt

                xt = emit_transposes(0, xbs)
                for g in range(NQG):
                    if g + 1 < NQG:  # prefetch next chunk
                        xbs = [load_xb(g + 1, b) for b in range(4)]
                    qsl = slice(g * 512, (g + 1) * 512)
                    for h in range(HPC):
                        hsl = slice(h * 128, (h + 1) * 128)
                        pq = ps_qk.tile([128, 512], F32, tag="pq")
                        for c in range(CB):
                            nc.tensor.matmul(
                                pq, wq_sb[:, c, hsl], xt[:, c, :],
                                start=(c == 0), stop=(c == CB - 1),
                            )
                        nc.scalar.activation(
                            q_t[:, h, qsl], pq,
                            mybir.ActivationFunctionType.Identity,
                            bias=bq_sb[:, h : h + 1],
                        )
                        pk = ps_qk.tile([128, 512], F32, tag="pk")
                        for c in range(CB):
                            nc.tensor.matmul(
                                pk, wk_sb[:, c, hsl], xt[:, c, :],
                                start=(c == 0), stop=(c == CB - 1),
                            )
                        nc.scalar.activation(
                            k_t[:, h, qsl], pk,
                            mybir.ActivationFunctionType.Identity,
                            bias=bk_sb[:, h : h + 1],
                        )
                    # transposes for the next chunk run on the PE here, so
                    # their PSUM->SBUF copies land while the V matmuls run
                    xt_next = emit_transposes(g + 1, xbs) if g + 1 < NQG else None
                    for b in range(4):
                        lb = g * 4 + b
                        pv = ps_v.tile([128, HG], F32)
                        for c in range(CB):
                            nc.tensor.matmul(
                                pv, xt[:, c, b * 128 : (b + 1) * 128], wv_sb[:, c, :],
                                start=(c == 0), stop=(c == CB - 1),
                            )
                        nc.vector.tensor_copy(v_t[:, lb, :], pv)
                    xt = xt_next

            # ---- phase 2: attention + output projection ----
            with (
                tc.tile_pool(name="attn_sb", bufs=1) as pa,
                tc.tile_pool(name="at_pool", bufs=8) as pat,
                tc.tile_pool(name="nrm_sb", bufs=3) as pn,
                tc.tile_pool(name="y_pool", bufs=3) as py_,
                tc.tile_pool(name="ps_s", bufs=2, space="PSUM") as ps_s,
                tc.tile_pool(name="ps_o", bufs=2, space="PSUM") as ps_o,
                tc.tile_pool(name="ps_n", bufs=1, space="PSUM") as ps_n,
                tc.tile_pool(name="ps_y", bufs=1, space="PSUM") as ps_y,
            ):
                # maskf[p, c] = 1.0 if c >= p + 384 else 0.0; diagonal-block
                # mask for block i (0..3) is maskf[:, 384-128i : 896-128i].
                # maskf[:, 768:896] is all-ones: also used as the stationary
                # of the broadcast row-sum matmuls. Loaded here, off the
                # startup critical path.
                nc.sync.dma_start(maskf, maskf_d.bitcast(F32R))
                wo_sb = pa.tile([128, HPC, D], F32R)
                nc.sync.dma_start(
                    wo_sb, wo_d.bitcast(F32R).rearrange("(h p) e -> p h e", p=128)
                )
                # Flat software-pipelined stream over all (g, h, j) batches.
                # Per batch: S-matmuls -> exp (ACT) -> mask (DVE, diag only)
                # -> PV + row-sum matmuls. The S-matmuls of batch m+1 are
                # emitted before the PV of batch m, so the PE queue always
                # has an exp-independent batch in front of it, and the
                # finalize / projection work (which trails DVE results) is
                # emitted a batch or two late to avoid head-of-line blocks.
                flat = []
                for g in range(NQG):
                    nb = 2 * (g + 1)
                    order = list(range(nb))
                    for h in range(HPC):
                        for pos, j in enumerate(order):
                            flat.append((g, h, j, pos == nb - 1, pos == 0))
                state = {}
                pending = []  # (delay, closure)

                def emit_S(m):
                    g, h, j, last, first = flat[m]
                    ps = ps_s.tile([128, 2, 512], F32, tag="ps")
                    for t in range(2):
                        kb = 2 * j + t
                        i = kb - 4 * g
                        # diag block i: q-cols < 128i are fully masked -- skip
                        # them, but keep N >= 256 (fp32r below 256 drops to
                        # 4 cyc/row, costing more than the dead columns)
                        c0 = min(128 * i, 256) if i > 0 else 0
                        nc.tensor.matmul(
                            ps[:, t, c0:],
                            k_t[:, h, kb * 128 : (kb + 1) * 128],
                            q_t[:, h, g * 512 + c0 : (g + 1) * 512],
                            start=True, stop=True,
                        )
                    state[m] = ps

                def emit_rest(m):
                    g, h, j, last, first = flat[m]
                    ps = state.pop(m)
                    if first:
                        state[("po", g, h)] = ps_o.tile([128, 512], F32, tag="po", name="po")
                        state[("sm", g, h)] = ps_n.tile([128, 512], F32, tag="nrm", name="psums")
                    po = state[("po", g, h)]
                    psums = state[("sm", g, h)]
                    at = pat.tile([128, 2, 512], F32R)
                    diag = j >= 2 * g
                    if diag:
                        # per-t exp over just the computed columns
                        for t in range(2):
                            c0 = min(128 * (2 * j + t - 4 * g), 256)
                            nc.scalar.activation(
                                at[:, t, c0:], ps[:, t, c0:], EXP, scale=SCALE
                            )
                    elif last:
                        # split: halves the exp latency gating this group's
                        # finalize chain
                        nc.scalar.activation(at[:, 0, :], ps[:, 0, :], EXP, scale=SCALE)
                        nc.scalar.activation(at[:, 1, :], ps[:, 1, :], EXP, scale=SCALE)
                    else:
                        nc.scalar.activation(at, ps, EXP, scale=SCALE)
                    for t in range(2):
                        kb = 2 * j + t
                        i = kb - 4 * g
                        if i == 3:
                            # zero the computed-but-dead [256,384) plus the
                            # [384,512) triangle in one slice
                            nc.vector.tensor_mul(
                                at[:, t, 256:512], at[:, t, 256:512],
                                maskf[:, 256:512],
                            )
                        elif i >= 0:
                            # triangle mask on the diagonal 128-block; the
                            # dead cols below it are never computed or read
                            nc.vector.tensor_mul(
                                at[:, t, 128 * i : 128 * i + 128],
                                at[:, t, 128 * i : 128 * i + 128],
                                maskf[:, 384:512],
                            )
                        st, sp = first and t == 0, last and t == 1
                        c0 = min(128 * i, 256) if i > 0 else 0
                        nc.tensor.matmul(
                            po[:, c0:],
                            v_t[:, kb, h * 128 : (h + 1) * 128],
                            at[:, t, c0:],
                            start=st, stop=sp,
                        )
                        nc.tensor.matmul(
                            psums[:, c0:],
                            maskf[:, 768:896],
                            at[:, t, c0:],
                            start=st, stop=sp,
                        )

                def emit_finalize(g, h):
                    def run():
                        po = state.pop(("po", g, h))
                        psums = state.pop(("sm", g, h))
                        recip = pn.tile([128, 512], F32, tag="recip")
                        nc.vector.reciprocal(recip, psums)
                        nc.vector.tensor_mul(
                            o_t[:, h, g * 512 : (g + 1) * 512], po, recip
                        )
                    return run

                def emit_proj(g):
                    def run():
                        # the last group's projection runs exposed after all
                        # attention work; borrow the then-idle ps_s slots to
                        # triple-buffer it
                        final = g == NQG - 1
                        for b in range(4):
                            lb = g * 4 + b
                            lsl = slice(lb * 128, (lb + 1) * 128)
                            ysb = py_.tile([128, D], F32, tag="ysb")
                            for eh in range(2):
                                pool = ps_s if final and (b + eh) % 2 else ps_y
                                pyp = pool.tile(
                                    [128, 384], F32,
                                    tag="ps" if pool is ps_s else "pyp",
                                    name="pyp",
                                )
                                for h in range(HPC):
                                    nc.tensor.matmul(
                                        pyp,
                                        o_t[:, h, lsl],
                                        wo_sb[:, h, eh * 384 : (eh + 1) * 384],
                                        start=(h == 0), stop=(h == HPC - 1),
                                    )
                                nc.vector.tensor_copy(
                                    ysb[:, eh * 384 : (eh + 1) * 384], pyp
                                )
                            nc.sync.dma_start(y_d[lb * 128 : (lb + 1) * 128, :], ysb)
                    return run

                emit_S(0)
                for m in range(len(flat)):
                    if m + 1 < len(flat):
                        emit_S(m + 1)
                    nxt = []
                    for d, fn in pending:
                        if d <= 0:
                            fn()
                        else:
                            nxt.append((d - 1, fn))
                    pending = nxt
                    emit_rest(m)
                    g, h, j, last, first = flat[m]
                    if last:
                        pending.append((1, emit_finalize(g, h)))
                        if h == HPC - 1:
                            pending.append((2, emit_proj(g)))
                for d, fn in sorted(pending, key=lambda p: p[0]):
                    fn()

    nc.compile()
    return nc


_NC_CACHE = {}


def _get_nc(L_=L):
    if L_ not in _NC_CACHE:
        _NC_CACHE[L_] = build_nc(L_)
    return _NC_CACHE[L_]


def run_sharded(inputs, L_=L, trace=False):
    """Shard inputs over 8 cores, run, return (list of per-core y, results obj)."""
    x = np.ascontiguousarray(inputs["x_input"], dtype=np.float32)
    ident = np.eye(128, dtype=np.float32)
    maskf = (np.arange(896)[None, :] >= np.arange(128)[:, None] + 384).astype(
        np.float32
    )
    in_maps = []
    for c in range(N_CORES):
        b, gslice = c // 2, slice((c % 2) * HG, (c % 2) * HG + HG)
        in_maps.append(
            {
                "x": x[b],
                "ident": ident,
                "maskf": maskf,
                "wq": np.ascontiguousarray(inputs["Wq"][:, gslice], np.float32),
                "wk": np.ascontiguousarray(inputs["Wk"][:, gslice], np.float32),
                "wv": np.ascontiguousarray(inputs["Wv"][:, gslice], np.float32),
                "wo": np.ascontiguousarray(inputs["Wo"][gslice, :], np.float32),
                "bq": np.ascontiguousarray(inputs["bq"][gslice], np.float32),
                "bk": np.ascontiguousarray(inputs["bk"][gslice], np.float32),
            }
        )
    nc = _get_nc(L_)
    try:
        res = run_bass_kernel_spmd(nc, in_maps, list(range(N_CORES)), trace=trace)
    except Exception:
        # transient device faults (NRT_EXEC_UNIT_UNRECOVERABLE etc.): one retry
        res = run_bass_kernel_spmd(nc, in_maps, list(range(N_CORES)), trace=trace)
    return res


def kernel(**inputs) -> np.ndarray:
    res = run_sharded(inputs)
    # host-side unshard: sum the two head-group partials per batch; add the
    # bias terms that commute out of the device computation exactly:
    # softmax rows sum to 1, so  A @ (xWv + bv) Wo + bo = A(xWv)Wo + bv@Wo + bo
    bias = (
        np.asarray(inputs["bv"], np.float32) @ np.asarray(inputs["Wo"], np.float32)
        + np.asarray(inputs["bo"], np.float32)
    )
    out = np.empty((B, L, D), dtype=np.float32)
    for b in range(B):
        out[b] = res.results[2 * b]["y"] + res.results[2 * b + 1]["y"] + bias
    return out



# revision 2
# speedup vs baseline: 1.1075x; 1.1075x over previous
"""Causal self-attention for B=4, L=2048, D=768, H=6 on 8 TRN2 NeuronCores.

Sharding: 8 cores = 4 batches x 2 head-groups (3 heads / 384 hidden each).
All matmul operands are fp16 (host converts x/weights; ~0.4% rel err, well
inside the 2e-2 gate). Per core, for its (batch, head-group):

  x^T is uploaded pre-transposed (fp16), so no PE transposes at all.
  QT/KT = (Wq,k chunk)^T-stationary @ x^T-moving   [128d x L per head]
  V     = x^T-stationary @ Wv-moving               [L x 384]
  per head, per 512-wide q-group, per 128-key block (causal skip at 128
  granularity — fp16 runs 1 cyc/row at any width):
    S^T  = K_blk @ Q^T            (PE)
    A^T  = exp(S^T/sqrt(128) - 2) (ACT, fp16 out; -2 guards fp16 range)
    tri-mask on diagonal blocks   (DVE, fp16 2x mode)
    O^T += V_blk^T @ A^T          (PE, accumulated in PSUM)
    Bsum += A^T                   (DVE fp16 adds — replaces the row-sum
                                   ones-matmuls that used to burn PE time)
  sums  = partition_all_reduce(Bsum)  (GPSIMD/Pool — idle engine)
  recip = 1/sums; O^T *= recip        (DVE)
  Y_part = O @ Wo_slice               (PE, via O^T-stationary)
Host sums the two head-group partials per batch and adds (bv @ Wo + bo);
bq/bk are applied on-device (free per-partition bias in the PSUM->SBUF
copies). The exp -2 bias cancels in softmax normalization exactly.
"""

import math

import numpy as np

import concourse.bacc as bacc
import concourse.mybir as mybir
import concourse.tile as tile
from concourse import bass_isa
from concourse.bass_utils import run_bass_kernel_spmd

F32 = mybir.dt.float32
F16 = mybir.dt.float16
EXP = mybir.ActivationFunctionType.Exp
IDENT = mybir.ActivationFunctionType.Identity

B = 4
L = 2048
D = 768
HEADS = 6
HD = 128
HPC = 3          # heads per core
HG = HPC * HD    # 384: per-core slice of the hidden dim
CB = D // 128    # 6 contraction chunks
SCALE = 1.0 / math.sqrt(HD)
EXP_BIAS = -2.0  # exp(S*scale - 2): keeps A and its sums in fp16 range
N_CORES = 8


def build_nc(L_=L):
    """Build + compile the per-core Bass program (same program on all cores)."""
    NQG = L_ // 512   # 512-wide q groups

    nc = bacc.Bacc("TRN2", target_bir_lowering=False, debug=False)
    xt_d = nc.dram_tensor("xt", [D, L_], F16, kind="ExternalInput").ap()
    wqkv_d = nc.dram_tensor("wqkv", [D, 3 * HG], F16, kind="ExternalInput").ap()
    wo_d = nc.dram_tensor("wo", [HG, D], F16, kind="ExternalInput").ap()
    bq_d = nc.dram_tensor("bq", [HG], F32, kind="ExternalInput").ap()
    bk_d = nc.dram_tensor("bk", [HG], F32, kind="ExternalInput").ap()
    tri_d = nc.dram_tensor("tri", [128, 128], F16, kind="ExternalInput").ap()
    eb_d = nc.dram_tensor("eb", [128, 1], F32, kind="ExternalInput").ap()
    y_d = nc.dram_tensor("y", [L_, D], F32, kind="ExternalOutput").ap()

    with tile.TileContext(nc) as tc:
        with (
            tc.tile_pool(name="persist", bufs=1) as pp,
            tc.tile_pool(name="qkv_sb", bufs=1) as pqkv,
        ):
            # constants go on the SWDGE (gpsimd) queue so the HWDGE queue's
            # first descriptors are the x^T / weight chunks the PE waits on
            tri = pp.tile([128, 128], F16)
            nc.gpsimd.dma_start(tri, tri_d)
            eb = pp.tile([128, 1], F32)
            nc.gpsimd.dma_start(eb, eb_d)
            bq_sb = pp.tile([128, HPC], F32)
            bk_sb = pp.tile([128, HPC], F32)
            nc.gpsimd.dma_start(bq_sb, bq_d.rearrange("(h p) -> p h", p=128))
            nc.gpsimd.dma_start(bk_sb, bk_d.rearrange("(h p) -> p h", p=128))
            # dummy exp: pulls the ACT Exp-table load off the QKV->attention
            # transition and into the startup DMA shadow
            warm = pp.tile([1, 1], F32)
            nc.scalar.activation(warm, eb[:1, :], EXP, bias=eb[:1, :])

            q_t = pqkv.tile([128, HPC, L_], F16)   # Q^T: [d, (head, L)]
            k_t = pqkv.tile([128, HPC, L_], F16)   # K^T
            v_t = pqkv.tile([128, L_ // 128, HG], F16)  # V: [k-in-block, (block, hd)]
            o_t = pqkv.tile([128, HPC, L_], F16)   # O^T (normalized)
            xt = pqkv.tile([128, CB, L_], F16)     # x^T: [d-in-chunk, (chunk, L)]
            wqkv_sb = pqkv.tile([128, CB, 3 * HG], F16)
            wo_sb = pqkv.tile([128, HPC, D], F16)

            xt_r = xt_d.rearrange("(c p) l -> p c l", p=128)
            wqkv_r = wqkv_d.rearrange("(c p) d -> p c d", p=128)
            # interleave so the group-0 Q/K matmuls can start ~2.5us in:
            # per chunk c, its first-512 x^T columns plus the full weight chunk
            for c in range(CB):
                nc.sync.dma_start(xt[:, c, 0:512], xt_r[:, c, 0:512])
                nc.sync.dma_start(wqkv_sb[:, c, :], wqkv_r[:, c, :])
            for c in range(CB):
                nc.sync.dma_start(xt[:, c, 512:L_], xt_r[:, c, 512:L_])
            nc.sync.dma_start(
                wo_sb, wo_d.rearrange("(h p) e -> p h e", p=128)
            )

            # ---- phase 1: QKV projections ----
            with (
                tc.tile_pool(name="ps_qk", bufs=2, space="PSUM") as ps_qk,
                tc.tile_pool(name="ps_v", bufs=2, space="PSUM") as ps_v,
            ):
                for g in range(NQG):
                    qsl = slice(g * 512, (g + 1) * 512)
                    for h in range(HPC):
                        hsl = slice(h * 128, (h + 1) * 128)
                        pq = ps_qk.tile([128, 512], F32, tag="pq")
                        for c in range(CB):
                            nc.tensor.matmul(
                                pq, wqkv_sb[:, c, hsl], xt[:, c, qsl],
                                start=(c == 0), stop=(c == CB - 1),
                            )
                        nc.scalar.activation(
                            q_t[:, h, qsl], pq, IDENT, bias=bq_sb[:, h : h + 1]
                        )
                        pk = ps_qk.tile([128, 512], F32, tag="pk")
                        for c in range(CB):
                            nc.tensor.matmul(
                                pk, wqkv_sb[:, c, HG + h * 128 : HG + (h + 1) * 128],
                                xt[:, c, qsl],
                                start=(c == 0), stop=(c == CB - 1),
                            )
                        nc.scalar.activation(
                            k_t[:, h, qsl], pk, IDENT, bias=bk_sb[:, h : h + 1]
                        )
                    for b in range(4):
                        lb = g * 4 + b
                        pv = ps_v.tile([128, HG], F32)
                        for c in range(CB):
                            nc.tensor.matmul(
                                pv, xt[:, c, lb * 128 : (lb + 1) * 128],
                                wqkv_sb[:, c, 2 * HG : 3 * HG],
                                start=(c == 0), stop=(c == CB - 1),
                            )
                        nc.vector.tensor_copy(v_t[:, lb, :], pv)

            # ---- phase 2: attention + output projection ----
            with (
                tc.tile_pool(name="at_pool", bufs=8) as pat,
                tc.tile_pool(name="bsum_p", bufs=3) as pbs,
                tc.tile_pool(name="nrm_sb", bufs=3) as pn,
                tc.tile_pool(name="y_pool", bufs=3) as py_,
                tc.tile_pool(name="ps_s", bufs=2, space="PSUM") as ps_s,
                tc.tile_pool(name="ps_o", bufs=2, space="PSUM") as ps_o,
                tc.tile_pool(name="ps_y", bufs=2, space="PSUM") as ps_y,
            ):
                # Flat software-pipelined stream over all (g, h, j) batches.
                # Per batch: S-matmuls -> exp (ACT) -> mask (DVE, diag only)
                # -> PV matmuls (PE) + Bsum accumulation (DVE). The S-matmuls
                # of batch m+1 are emitted before the PV of batch m, so the PE
                # queue always has an exp-independent batch in front of it.
                # Finalize (Pool reduce + recip + normalize) and projection
                # trail by 1-2 batches.
                flat = []
                for g in range(NQG):
                    nb = 2 * (g + 1)
                    for h in range(HPC):
                        for pos in range(nb):
                            flat.append((g, h, pos, pos == nb - 1, pos == 0))
                state = {}
                pending = []  # (delay, closure)

                def emit_S(m):
                    g, h, j, last, first = flat[m]
                    ps = ps_s.tile([128, 2, 512], F32, tag="ps")
                    for t in range(2):
                        kb = 2 * j + t
                        i = kb - 4 * g
                        c0 = 128 * i if i > 0 else 0
                        nc.tensor.matmul(
                            ps[:, t, c0:],
                            k_t[:, h, kb * 128 : (kb + 1) * 128],
                            q_t[:, h, g * 512 + c0 : (g + 1) * 512],
                            start=True, stop=True,
                        )
                    state[m] = ps

                def emit_rest(m):
                    g, h, j, last, first = flat[m]
                    ps = state.pop(m)
                    if first:
                        state[("po", g, h)] = ps_o.tile(
                            [128, 512], F32, tag="po", name="po"
                        )
                        state[("bs", g, h)] = pbs.tile(
                            [128, 512], F16, tag="bs", name="bsum"
                        )
                    po = state[("po", g, h)]
                    bsum = state[("bs", g, h)]
                    at = pat.tile([128, 2, 512], F16)
                    diag = j >= 2 * g
                    if diag:
                        # per-t exp over just the computed columns
                        for t in range(2):
                            c0 = 128 * (2 * j + t - 4 * g)
                            nc.scalar.activation(
                                at[:, t, c0:], ps[:, t, c0:], EXP,
                                scale=SCALE, bias=eb,
                            )
                    elif last:
                        # split: halves the exp latency gating this group's
                        # finalize chain
                        nc.scalar.activation(
                            at[:, 0, :], ps[:, 0, :], EXP, scale=SCALE, bias=eb
                        )
                        nc.scalar.activation(
                            at[:, 1, :], ps[:, 1, :], EXP, scale=SCALE, bias=eb
                        )
                    else:
                        nc.scalar.activation(at, ps, EXP, scale=SCALE, bias=eb)
                    for t in range(2):
                        kb = 2 * j + t
                        i = kb - 4 * g
                        c0 = 128 * i if i > 0 else 0
                        if i >= 0:
                            # triangle mask on the diagonal 128-block; columns
                            # left of it are never computed or read
                            nc.vector.tensor_mul(
                                at[:, t, c0 : c0 + 128],
                                at[:, t, c0 : c0 + 128],
                                tri,
                            )
                        # Bsum accumulation on DVE (fp16 2x) replaces the
                        # ones-matmul row sums
                        if first and t == 0:
                            nc.vector.tensor_copy(bsum, at[:, 0, :])
                        else:
                            nc.vector.tensor_add(
                                bsum[:, c0:], bsum[:, c0:], at[:, t, c0:]
                            )
                        st, sp = first and t == 0, last and t == 1
                        nc.tensor.matmul(
                            po[:, c0:],
                            v_t[:, kb, h * 128 : (h + 1) * 128],
                            at[:, t, c0:],
                            start=st, stop=sp,
                        )

                def emit_finalize(g, h):
                    def run():
                        po = state.pop(("po", g, h))
                        bsum = state.pop(("bs", g, h))
                        sums = pn.tile([128, 512], F32, tag="sums")
                        nc.gpsimd.partition_all_reduce(
                            sums, bsum, 128, bass_isa.ReduceOp.add
                        )
                        recip = pn.tile([128, 512], F32, tag="recip")
                        nc.vector.reciprocal(recip, sums)
                        nc.vector.tensor_mul(
                            o_t[:, h, g * 512 : (g + 1) * 512], po, recip
                        )
                    return run

                def emit_proj(g):
                    def run():
                        # the last group's projection runs exposed after all
                        # attention work; borrow the then-idle ps_s slots to
                        # quad-buffer it
                        final = g == NQG - 1
                        for b in range(4):
                            lb = g * 4 + b
                            lsl = slice(lb * 128, (lb + 1) * 128)
                            ysb = py_.tile([128, D], F32, tag="ysb")
                            for eh in range(2):
                                pool = ps_y if not (final and (b + eh) % 2) else ps_s
                                pyp = pool.tile(
                                    [128, 384], F32,
                                    tag="pyp" if pool is ps_y else "ps",
                                    name="pyp",
                                )
                                for h in range(HPC):
                                    nc.tensor.matmul(
                                        pyp,
                                        o_t[:, h, lsl],
                                        wo_sb[:, h, eh * 384 : (eh + 1) * 384],
                                        start=(h == 0), stop=(h == HPC - 1),
                                    )
                                eng = nc.vector if eh == 0 else nc.scalar
                                if eh == 0:
                                    nc.vector.tensor_copy(
                                        ysb[:, eh * 384 : (eh + 1) * 384], pyp
                                    )
                                else:
                                    nc.scalar.activation(
                                        ysb[:, eh * 384 : (eh + 1) * 384], pyp,
                                        IDENT, bias=0.0,
                                    )
                            nc.sync.dma_start(y_d[lb * 128 : (lb + 1) * 128, :], ysb)
                    return run

                emit_S(0)
                for m in range(len(flat)):
                    if m + 1 < len(flat):
                        emit_S(m + 1)
                    nxt = []
                    for d, fn in pending:
                        if d <= 0:
                            fn()
                        else:
                            nxt.append((d - 1, fn))
                    pending = nxt
                    emit_rest(m)
                    g, h, j, last, first = flat[m]
                    if last:
                        pending.append((1, emit_finalize(g, h)))
                        if h == HPC - 1:
                            pending.append((2, emit_proj(g)))
                for d, fn in sorted(pending, key=lambda p: p[0]):
                    fn()

    nc.compile()
    return nc


_NC_CACHE = {}


def _get_nc(L_=L):
    if L_ not in _NC_CACHE:
        _NC_CACHE[L_] = build_nc(L_)
    return _NC_CACHE[L_]


def run_sharded(inputs, L_=L, trace=False):
    """Shard inputs over 8 cores, run, return results object."""
    x = np.asarray(inputs["x_input"], dtype=np.float32)
    tri = (np.arange(128)[None, :] >= np.arange(128)[:, None]).astype(np.float16)
    eb = np.full((128, 1), EXP_BIAS, dtype=np.float32)
    in_maps = []
    for c in range(N_CORES):
        b, gslice = c // 2, slice((c % 2) * HG, (c % 2) * HG + HG)
        wqkv = np.concatenate(
            [
                np.asarray(inputs["Wq"], np.float32)[:, gslice],
                np.asarray(inputs["Wk"], np.float32)[:, gslice],
                np.asarray(inputs["Wv"], np.float32)[:, gslice],
            ],
            axis=1,
        ).astype(np.float16)
        in_maps.append(
            {
                "xt": np.ascontiguousarray(x[b].T.astype(np.float16)),
                "wqkv": np.ascontiguousarray(wqkv),
                "wo": np.ascontiguousarray(
                    np.asarray(inputs["Wo"], np.float32)[gslice, :].astype(np.float16)
                ),
                "bq": np.ascontiguousarray(
                    np.asarray(inputs["bq"], np.float32)[gslice]
                ),
                "bk": np.ascontiguousarray(
                    np.asarray(inputs["bk"], np.float32)[gslice]
                ),
                "tri": tri,
                "eb": eb,
            }
        )
    nc = _get_nc(L_)
    try:
        res = run_bass_kernel_spmd(nc, in_maps, list(range(N_CORES)), trace=trace)
    except Exception:
        # transient device faults (NRT_EXEC_UNIT_UNRECOVERABLE etc.): one retry
        res = run_bass_kernel_spmd(nc, in_maps, list(range(N_CORES)), trace=trace)
    return res


def kernel(**inputs) -> np.ndarray:
    res = run_sharded(inputs)
    # host-side unshard: sum the two head-group partials per batch; add the
    # bias terms that commute out of the device computation exactly:
    # softmax rows sum to 1, so  A @ (xWv + bv) Wo + bo = A(xWv)Wo + bv@Wo + bo
    bias = (
        np.asarray(inputs["bv"], np.float32) @ np.asarray(inputs["Wo"], np.float32)
        + np.asarray(inputs["bo"], np.float32)
    )
    out = np.empty((B, L, D), dtype=np.float32)
    for b in range(B):
        out[b] = res.results[2 * b]["y"] + res.results[2 * b + 1]["y"] + bias
    return out


# revision 3
# speedup vs baseline: 1.2170x; 1.0990x over previous
"""Causal self-attention for B=4, L=2048, D=768, H=6 on 8 TRN2 NeuronCores.

Sharding: 8 cores = 4 batches x 2 head-groups (3 heads / 384 hidden each).
All matmul operands are fp16 (host converts x/weights; ~0.5% rel err, well
inside the 2e-2 gate). Per core, for its (batch, head-group):

  x^T is uploaded pre-transposed (fp16), so no PE transposes at all.
  QT/KT = (Wq,k chunk)^T-stationary @ x^T-moving   [128d x L per head]
  V     = x^T-stationary @ Wv-moving               [L x 384]
  per head, per 512-wide q-group, per 128-key block (causal skip at 128
  granularity — fp16 runs 1 cyc/row at any width):
    S^T  = K_blk @ Q^T            (PE)
    A^T  = exp(S^T/sqrt(128) - 2) (ACT, fp16 out; -2 guards fp16 range)
    tri-mask on diagonal blocks   (DVE, fp16 2x mode)
    O^T += V_blk^T @ A^T          (PE, accumulated in PSUM)
    Bsum += A^T                   (DVE fp16 adds — replaces the row-sum
                                   ones-matmuls that used to burn PE time)
  sums  = partition_all_reduce(Bsum)  (GPSIMD/Pool — idle engine)
  recip = 1/sums; O^T *= recip        (DVE)
  Y_part = O @ Wo_slice               (PE, via O^T-stationary)

The QKV projections are interleaved into the attention stream: the PE
executes group g+1's QKV matmuls between group g's attention batches, so
the exp (ACT) latency is hidden behind projection work instead of stalling
the PE.  Host sums the two head-group partials per batch and adds
(bv @ Wo + bo); bq/bk are applied on-device (free per-partition bias in
the PSUM->SBUF copies). The exp -2 bias cancels in softmax normalization.
"""

import math

import numpy as np

import concourse.bacc as bacc
import concourse.mybir as mybir
import concourse.tile as tile
from concourse import bass_isa
from concourse.bass_utils import run_bass_kernel_spmd

F32 = mybir.dt.float32
F16 = mybir.dt.float16
EXP = mybir.ActivationFunctionType.Exp
IDENT = mybir.ActivationFunctionType.Identity

B = 4
L = 2048
D = 768
HEADS = 6
HD = 128
HPC = 3          # heads per core
HG = HPC * HD    # 384: per-core slice of the hidden dim
CB = D // 128    # 6 contraction chunks
SCALE = 1.0 / math.sqrt(HD)
EXP_BIAS = -2.0  # exp(S*scale - 2): keeps A and its sums in fp16 range
N_CORES = 8


def build_nc(L_=L):
    """Build + compile the per-core Bass program (same program on all cores)."""
    NQG = L_ // 512   # 512-wide q groups

    nc = bacc.Bacc("TRN2", target_bir_lowering=False, debug=False)
    xt_d = nc.dram_tensor("xt", [D, L_], F16, kind="ExternalInput").ap()
    wqkv_d = nc.dram_tensor("wqkv", [D, 3 * HG], F16, kind="ExternalInput").ap()
    wo_d = nc.dram_tensor("wo", [HG, D], F16, kind="ExternalInput").ap()
    bq_d = nc.dram_tensor("bq", [HG], F32, kind="ExternalInput").ap()
    bk_d = nc.dram_tensor("bk", [HG], F32, kind="ExternalInput").ap()
    tri_d = nc.dram_tensor("tri", [128, 128], F16, kind="ExternalInput").ap()
    eb_d = nc.dram_tensor("eb", [128, 1], F32, kind="ExternalInput").ap()
    y_d = nc.dram_tensor("y", [L_, D], F32, kind="ExternalOutput").ap()

    with tile.TileContext(nc) as tc:
        with (
            tc.tile_pool(name="persist", bufs=1) as pp,
            tc.tile_pool(name="qkv_sb", bufs=1) as pqkv,
            tc.tile_pool(name="at_pool", bufs=8) as pat,
            tc.tile_pool(name="bsum_p", bufs=3) as pbs,
            tc.tile_pool(name="nrm_sb", bufs=3) as pn,
            tc.tile_pool(name="y_pool", bufs=3) as py_,
            tc.tile_pool(name="ps_s", bufs=2, space="PSUM") as ps_s,
            tc.tile_pool(name="ps_o", bufs=2, space="PSUM") as ps_o,
            tc.tile_pool(name="ps_sh", bufs=2, space="PSUM") as ps_sh,
        ):
            # constants go on the SWDGE (gpsimd) queue so the HWDGE queue's
            # first descriptors are the weight / x^T chunks the PE waits on
            eb = pp.tile([128, 1], F32)
            nc.gpsimd.dma_start(eb, eb_d)
            tri = pp.tile([128, 128], F16)
            nc.gpsimd.dma_start(tri, tri_d)
            bq_sb = pp.tile([128, HPC], F32)
            bk_sb = pp.tile([128, HPC], F32)
            nc.gpsimd.dma_start(bq_sb, bq_d.rearrange("(h p) -> p h", p=128))
            nc.gpsimd.dma_start(bk_sb, bk_d.rearrange("(h p) -> p h", p=128))
            # dummy exp: pulls the ACT Exp-table load off the critical path
            warm = pp.tile([1, 1], F32)
            nc.scalar.activation(warm, eb[:1, :], EXP, bias=eb[:1, :])

            q_t = pqkv.tile([128, HPC, L_], F16)   # Q^T: [d, (head, L)]
            k_t = pqkv.tile([128, HPC, L_], F16)   # K^T
            v_t = pqkv.tile([128, L_ // 128, HG], F16)  # V: [k-in-block, (block, hd)]
            o_t = pqkv.tile([128, HPC, L_], F16)   # O^T (normalized)
            xt = pqkv.tile([128, CB, L_], F16)     # x^T: [d-in-chunk, (chunk, L)]
            wqkv_sb = pqkv.tile([128, CB, 3 * HG], F16)
            wo_sb = pqkv.tile([128, HPC, D], F16)

            xt_r = xt_d.rearrange("(c p) l -> p c l", p=128)
            wqkv_r = wqkv_d.rearrange("(c p) d -> p c d", p=128)
            # interleave so the group-0 Q/K matmuls can start ~3us in:
            # per chunk c, the weight chunk plus the first-512 x^T columns
            for c in range(CB):
                nc.sync.dma_start(wqkv_sb[:, c, :], wqkv_r[:, c, :])
                nc.sync.dma_start(xt[:, c, 0:512], xt_r[:, c, 0:512])
            for c in range(CB):
                nc.sync.dma_start(xt[:, c, 512:L_], xt_r[:, c, 512:L_])
            nc.sync.dma_start(wo_sb, wo_d.rearrange("(h p) e -> p h e", p=128))

            # tiny matmul as soon as eb lands: starts the PE p-state ramp
            # clock ~1.5us before the first real matmul
            dmy = ps_sh.tile([128, 512], F32, tag="sh", name="dmy")
            nc.tensor.matmul(dmy[:1, :1], eb, eb, start=True, stop=True)

            # ---- QKV group 0: chunk-major so the PE rides the arriving
            # per-chunk DMAs without stalling (3 heads' q+k accumulate in
            # 6 PSUM banks at once; attention pools are idle this early) ----
            qk_ps = {
                0: (lambda t_: (t_[:, 0, :], t_[:, 1, :]))(
                    ps_s.tile([128, 2, 512], F32, tag="ps", name="qk0")
                ),
                1: (lambda t_: (t_[:, 0, :], t_[:, 1, :]))(
                    ps_s.tile([128, 2, 512], F32, tag="ps", name="qk1")
                ),
                2: (
                    ps_o.tile([128, 512], F32, tag="po", name="qk2q"),
                    ps_sh.tile([128, 512], F32, tag="sh", name="qk2k"),
                ),
            }
            for c in range(CB):
                for h in range(HPC):
                    pq, pk = qk_ps[h]
                    nc.tensor.matmul(
                        pq, wqkv_sb[:, c, h * 128 : (h + 1) * 128],
                        xt[:, c, 0:512],
                        start=(c == 0), stop=(c == CB - 1),
                    )
                    nc.tensor.matmul(
                        pk, wqkv_sb[:, c, HG + h * 128 : HG + (h + 1) * 128],
                        xt[:, c, 0:512],
                        start=(c == 0), stop=(c == CB - 1),
                    )
            for h in range(HPC):
                pq, pk = qk_ps[h]
                nc.scalar.activation(
                    q_t[:, h, 0:512], pq, IDENT, bias=bq_sb[:, h : h + 1]
                )
                nc.scalar.activation(
                    k_t[:, h, 0:512], pk, IDENT, bias=bk_sb[:, h : h + 1]
                )
            for b in range(4):
                pv = ps_sh.tile([128, 512], F32, tag="sh", name="pv")
                for c in range(CB):
                    nc.tensor.matmul(
                        pv[:, :HG], xt[:, c, b * 128 : (b + 1) * 128],
                        wqkv_sb[:, c, 2 * HG : 3 * HG],
                        start=(c == 0), stop=(c == CB - 1),
                    )
                nc.vector.tensor_copy(v_t[:, b, :], pv[:, :HG])

            # ---- interleaved QKV(g+1) / attention(g) stream ----

            def emit_qk_unit(g, h):
                qsl = slice(g * 512, (g + 1) * 512)
                pq = ps_sh.tile([128, 512], F32, tag="sh", name="pq")
                for c in range(CB):
                    nc.tensor.matmul(
                        pq, wqkv_sb[:, c, h * 128 : (h + 1) * 128], xt[:, c, qsl],
                        start=(c == 0), stop=(c == CB - 1),
                    )
                nc.scalar.activation(
                    q_t[:, h, qsl], pq, IDENT, bias=bq_sb[:, h : h + 1]
                )
                pk = ps_sh.tile([128, 512], F32, tag="sh", name="pk")
                for c in range(CB):
                    nc.tensor.matmul(
                        pk, wqkv_sb[:, c, HG + h * 128 : HG + (h + 1) * 128],
                        xt[:, c, qsl],
                        start=(c == 0), stop=(c == CB - 1),
                    )
                nc.scalar.activation(
                    k_t[:, h, qsl], pk, IDENT, bias=bk_sb[:, h : h + 1]
                )

            def emit_v_unit(g, b):
                lb = g * 4 + b
                pv = ps_sh.tile([128, 512], F32, tag="sh", name="pv")
                for c in range(CB):
                    nc.tensor.matmul(
                        pv[:, :HG], xt[:, c, lb * 128 : (lb + 1) * 128],
                        wqkv_sb[:, c, 2 * HG : 3 * HG],
                        start=(c == 0), stop=(c == CB - 1),
                    )
                nc.vector.tensor_copy(v_t[:, lb, :], pv[:, :HG])

            # attention batches: per (g,h), j indexes pairs of 128-key blocks
            flat = []
            win_start = {}
            for g in range(NQG):
                win_start[g] = len(flat)
                nb = 2 * (g + 1)
                for h in range(HPC):
                    for pos in range(nb):
                        flat.append((g, h, pos, pos == nb - 1, pos == 0))
            state = {}
            pending = []  # (delay, closure)

            def emit_S(m):
                g, h, j, last, first = flat[m]
                ps = ps_s.tile([128, 2, 512], F32, tag="ps")
                for t in range(2):
                    kb = 2 * j + t
                    i = kb - 4 * g
                    c0 = 128 * i if i > 0 else 0
                    nc.tensor.matmul(
                        ps[:, t, c0:],
                        k_t[:, h, kb * 128 : (kb + 1) * 128],
                        q_t[:, h, g * 512 + c0 : (g + 1) * 512],
                        start=True, stop=True,
                    )
                state[m] = ps

            def emit_rest(m):
                g, h, j, last, first = flat[m]
                ps = state.pop(m)
                if first:
                    state[("po", g, h)] = ps_o.tile(
                        [128, 512], F32, tag="po", name="po"
                    )
                    state[("bs", g, h)] = pbs.tile(
                        [128, 512], F16, tag="bs", name="bsum"
                    )
                po = state[("po", g, h)]
                bsum = state[("bs", g, h)]
                at = pat.tile([128, 2, 512], F16)
                diag = j >= 2 * g
                if diag:
                    # per-t exp over just the computed columns
                    for t in range(2):
                        c0 = 128 * (2 * j + t - 4 * g)
                        nc.scalar.activation(
                            at[:, t, c0:], ps[:, t, c0:], EXP,
                            scale=SCALE, bias=eb,
                        )
                elif last:
                    # split: halves the exp latency gating this group's
                    # finalize chain
                    nc.scalar.activation(
                        at[:, 0, :], ps[:, 0, :], EXP, scale=SCALE, bias=eb
                    )
                    nc.scalar.activation(
                        at[:, 1, :], ps[:, 1, :], EXP, scale=SCALE, bias=eb
                    )
                else:
                    nc.scalar.activation(at, ps, EXP, scale=SCALE, bias=eb)
                for t in range(2):
                    kb = 2 * j + t
                    i = kb - 4 * g
                    c0 = 128 * i if i > 0 else 0
                    if i >= 0:
                        # triangle mask on the diagonal 128-block; columns
                        # left of it are never computed or read
                        nc.vector.tensor_mul(
                            at[:, t, c0 : c0 + 128], at[:, t, c0 : c0 + 128], tri
                        )
                    # Bsum accumulation on DVE (fp16 2x) replaces the
                    # ones-matmul row sums
                    if first and t == 0:
                        nc.vector.tensor_copy(bsum, at[:, 0, :])
                    else:
                        nc.vector.tensor_add(
                            bsum[:, c0:], bsum[:, c0:], at[:, t, c0:]
                        )
                    st, sp = first and t == 0, last and t == 1
                    nc.tensor.matmul(
                        po[:, c0:],
                        v_t[:, kb, h * 128 : (h + 1) * 128],
                        at[:, t, c0:],
                        start=st, stop=sp,
                    )
                if last:
                    # cross-partition reduce launches immediately (idle Pool
                    # engine); recip/normalize trail by a batch so the DVE
                    # never head-blocks on the Pool semaphore
                    sums = pn.tile([128, 512], F32, tag="sums")
                    nc.gpsimd.partition_all_reduce(
                        sums, bsum, 128, bass_isa.ReduceOp.add
                    )
                    state[("sm", g, h)] = sums

            def emit_finalize(g, h):
                def run():
                    po = state.pop(("po", g, h))
                    state.pop(("bs", g, h))
                    sums = state.pop(("sm", g, h))
                    recip = pn.tile([128, 512], F32, tag="recip")
                    nc.vector.reciprocal(recip, sums)
                    nc.vector.tensor_mul(
                        o_t[:, h, g * 512 : (g + 1) * 512], po, recip
                    )
                return run

            def emit_proj(g):
                def run():
                    for b in range(4):
                        lb = g * 4 + b
                        lsl = slice(lb * 128, (lb + 1) * 128)
                        ysb = py_.tile([128, D], F32, tag="ysb")
                        for eh in range(2):
                            pyp = ps_sh.tile(
                                [128, 512], F32, tag="sh", name="pyp"
                            )
                            for h in range(HPC):
                                nc.tensor.matmul(
                                    pyp[:, :384],
                                    o_t[:, h, lsl],
                                    wo_sb[:, h, eh * 384 : (eh + 1) * 384],
                                    start=(h == 0), stop=(h == HPC - 1),
                                )
                            if eh == 0:
                                nc.vector.tensor_copy(
                                    ysb[:, 0:384], pyp[:, :384]
                                )
                            else:
                                nc.scalar.activation(
                                    ysb[:, 384:768], pyp[:, :384], IDENT,
                                    bias=0.0,
                                )
                        nc.sync.dma_start(y_d[lb * 128 : (lb + 1) * 128, :], ysb)
                return run

            # per-window QKV filler units: window g interleaves group g+1's
            # QKV work between group g's attention batches
            def window_units(g):
                if g + 1 >= NQG:
                    return []
                u = [("qk", g + 1, h) for h in range(HPC)]
                u += [("v", g + 1, b) for b in range(4)]
                return u

            def emit_unit(u):
                kind, g, i = u
                if kind == "qk":
                    emit_qk_unit(g, i)
                else:
                    emit_v_unit(g, i)

            emit_S(0)
            for g in range(NQG):
                units = window_units(g)
                nbatch = 6 * (g + 1)
                emitted_u = 0
                for bi in range(nbatch):
                    m = win_start[g] + bi
                    if m + 1 < len(flat):
                        emit_S(m + 1)
                    # spread this window's QKV units across its batches
                    want = (len(units) * (bi + 1)) // nbatch
                    while emitted_u < want:
                        emit_unit(units[emitted_u])
                        emitted_u += 1
                    nxt = []
                    for d, fn in pending:
                        if d <= 0:
                            fn()
                        else:
                            nxt.append((d - 1, fn))
                    pending = nxt
                    emit_rest(m)
                    _, h, j, last, first = flat[m]
                    if last:
                        pending.append((1, emit_finalize(g, h)))
                        if h == HPC - 1:
                            pending.append((2, emit_proj(g)))
            for d, fn in sorted(pending, key=lambda p: p[0]):
                fn()

    nc.compile()
    return nc


_NC_CACHE = {}


def _get_nc(L_=L):
    if L_ not in _NC_CACHE:
        _NC_CACHE[L_] = build_nc(L_)
    return _NC_CACHE[L_]


def run_sharded(inputs, L_=L, trace=False):
    """Shard inputs over 8 cores, run, return results object."""
    x = np.asarray(inputs["x_input"], dtype=np.float32)
    tri = (np.arange(128)[None, :] >= np.arange(128)[:, None]).astype(np.float16)
    eb = np.full((128, 1), EXP_BIAS, dtype=np.float32)
    in_maps = []
    for c in range(N_CORES):
        b, gslice = c // 2, slice((c % 2) * HG, (c % 2) * HG + HG)
        wqkv = np.concatenate(
            [
                np.asarray(inputs["Wq"], np.float32)[:, gslice],
                np.asarray(inputs["Wk"], np.float32)[:, gslice],
                np.asarray(inputs["Wv"], np.float32)[:, gslice],
            ],
            axis=1,
        ).astype(np.float16)
        in_maps.append(
            {
                "xt": np.ascontiguousarray(x[b].T.astype(np.float16)),
                "wqkv": np.ascontiguousarray(wqkv),
                "wo": np.ascontiguousarray(
                    np.asarray(inputs["Wo"], np.float32)[gslice, :].astype(np.float16)
                ),
                "bq": np.ascontiguousarray(
                    np.asarray(inputs["bq"], np.float32)[gslice]
                ),
                "bk": np.ascontiguousarray(
                    np.asarray(inputs["bk"], np.float32)[gslice]
                ),
                "tri": tri,
                "eb": eb,
            }
        )
    nc = _get_nc(L_)
    try:
        res = run_bass_kernel_spmd(nc, in_maps, list(range(N_CORES)), trace=trace)
    except Exception:
        # transient device faults (NRT_EXEC_UNIT_UNRECOVERABLE etc.): one retry
        res = run_bass_kernel_spmd(nc, in_maps, list(range(N_CORES)), trace=trace)
    return res


def kernel(**inputs) -> np.ndarray:
    res = run_sharded(inputs)
    # host-side unshard: sum the two head-group partials per batch; add the
    # bias terms that commute out of the device computation exactly:
    # softmax rows sum to 1, so  A @ (xWv + bv) Wo + bo = A(xWv)Wo + bv@Wo + bo
    bias = (
        np.asarray(inputs["bv"], np.float32) @ np.asarray(inputs["Wo"], np.float32)
        + np.asarray(inputs["bo"], np.float32)
    )
    out = np.empty((B, L, D), dtype=np.float32)
    for b in range(B):
        out[b] = res.results[2 * b]["y"] + res.results[2 * b + 1]["y"] + bias
    return out


# revision 6
# speedup vs baseline: 1.2850x; 1.0558x over previous
"""Causal self-attention for B=4, L=2048, D=768, H=6 on 8 TRN2 NeuronCores.

Sharding: 8 cores = 4 batches x 2 head-groups (3 heads / 384 hidden each).
All matmul operands are fp16 (host converts x/weights; ~0.5% rel err, well
inside the 2e-2 gate). Per core, for its (batch, head-group):

  x^T is uploaded pre-transposed (fp16), so no PE transposes at all.
  QT/KT = (Wq,k chunk)^T-stationary @ x^T-moving   [128d x L per head]
  V     = x^T-stationary @ Wv-moving               [L x 384]
  per head, per 512-wide q-group, per 128-key block (causal skip at 128
  granularity — fp16 runs 1 cyc/row at any width):
    S^T  = K_blk @ Q^T            (PE)
    A^T  = exp(S^T/sqrt(128) - 2) (ACT, fp16 out; -2 guards fp16 range)
    tri-mask on diagonal blocks   (DVE, fp16 2x mode)
    O^T += V_blk^T @ A^T          (PE, accumulated in PSUM)
    Bsum += A^T                   (DVE fp16 adds — replaces the row-sum
                                   ones-matmuls that used to burn PE time)
  sums  = partition_all_reduce(Bsum)  (GPSIMD/Pool — idle engine; the very
          last group instead uses a PE ones-matmul to cut tail latency)
  O^T   = po / sums                   (single DVE divide)
  Y_part = O @ Wo_slice               (PE, via O^T-stationary)

The QKV projections are interleaved into the attention stream: the PE
executes group g+1's QKV matmuls between group g's attention batches, so
the exp (ACT) latency is hidden behind projection work instead of stalling
the PE.  Host sums the two head-group partials per batch and adds
(bv @ Wo + bo); bq/bk are applied on-device (free per-partition bias in
the PSUM->SBUF copies). The exp -2 bias cancels in softmax normalization.
"""

import math

import numpy as np

import concourse.bacc as bacc
import concourse.mybir as mybir
import concourse.tile as tile
from concourse import bass_isa
from concourse.bass_utils import run_bass_kernel_spmd

F32 = mybir.dt.float32
F16 = mybir.dt.float16
EXP = mybir.ActivationFunctionType.Exp
IDENT = mybir.ActivationFunctionType.Identity
DIV = mybir.AluOpType.divide

B = 4
L = 2048
D = 768
HEADS = 6
HD = 128
HPC = 3          # heads per core
HG = HPC * HD    # 384: per-core slice of the hidden dim
CB = D // 128    # 6 contraction chunks
SCALE = 1.0 / math.sqrt(HD)
EXP_BIAS = -2.0  # exp(S*scale - 2): keeps A and its sums in fp16 range
N_CORES = 8


def build_nc(L_=L):
    """Build + compile the per-core Bass program (same program on all cores)."""
    NQG = L_ // 512   # 512-wide q groups

    nc = bacc.Bacc("TRN2", target_bir_lowering=False, debug=False)
    xt_d = nc.dram_tensor("xt", [D, L_], F16, kind="ExternalInput").ap()
    wqkv_d = nc.dram_tensor("wqkv", [D, 3 * HG], F16, kind="ExternalInput").ap()
    wo_d = nc.dram_tensor("wo", [HG, D], F16, kind="ExternalInput").ap()
    bq_d = nc.dram_tensor("bq", [HG], F32, kind="ExternalInput").ap()
    bk_d = nc.dram_tensor("bk", [HG], F32, kind="ExternalInput").ap()
    tri_d = nc.dram_tensor("tri", [128, 256], F16, kind="ExternalInput").ap()
    eb_d = nc.dram_tensor("eb", [128, 1], F32, kind="ExternalInput").ap()
    y_d = nc.dram_tensor("y", [L_, D], F16, kind="ExternalOutput").ap()

    with tile.TileContext(nc) as tc:
        with (
            tc.tile_pool(name="persist", bufs=1) as pp,
            tc.tile_pool(name="qkv_sb", bufs=1) as pqkv,
            tc.tile_pool(name="at_pool", bufs=8) as pat,
            tc.tile_pool(name="bsum_p", bufs=3) as pbs,
            tc.tile_pool(name="nrm_sb", bufs=3) as pn,
            tc.tile_pool(name="y_pool", bufs=3) as py_,
            tc.tile_pool(name="ps_s", bufs=2, space="PSUM") as ps_s,
            tc.tile_pool(name="ps_o", bufs=2, space="PSUM") as ps_o,
            tc.tile_pool(name="ps_sh", bufs=2, space="PSUM") as ps_sh,
        ):
            # tiny memset-fed matmul right at program start: begins the PE
            # p-state ramp clock ~3us before the first real matmul, so QKV
            # group 0 runs at full clock
            dseed = pp.tile([128, 2], F16)
            nc.vector.memset(dseed, 0)
            dmy = ps_sh.tile([128, 512], F32, tag="sh", name="dmy")
            nc.tensor.matmul(
                dmy[:1, :2], dseed[:, :1], dseed, start=True, stop=True
            )

            # constants go on the SWDGE (gpsimd) queue so the HWDGE queue's
            # first descriptors are the weight / x^T chunks the PE waits on
            eb = pp.tile([128, 1], F32)
            nc.gpsimd.dma_start(eb, eb_d)
            trio = pp.tile([128, 256], F16)
            nc.gpsimd.dma_start(trio, tri_d)
            bq_sb = pp.tile([128, HPC], F32)
            bk_sb = pp.tile([128, HPC], F32)
            nc.gpsimd.dma_start(bq_sb, bq_d.rearrange("(h p) -> p h", p=128))
            nc.gpsimd.dma_start(bk_sb, bk_d.rearrange("(h p) -> p h", p=128))
            # dummy exp: pulls the ACT Exp-table load off the critical path
            warm = pp.tile([1, 1], F32)
            nc.scalar.activation(warm, eb[:1, :], EXP, bias=eb[:1, :])

            q_t = pqkv.tile([128, HPC, L_], F16)   # Q^T: [d, (head, L)]
            k_t = pqkv.tile([128, HPC, L_], F16)   # K^T
            v_t = pqkv.tile([128, L_ // 128, HG], F16)  # V: [k-in-block, (block, hd)]
            o_t = pqkv.tile([128, HPC, L_], F16)   # O^T (normalized)
            xt = pqkv.tile([128, CB, L_], F16)     # x^T: [d-in-chunk, (chunk, L)]
            wqkv_sb = pqkv.tile([128, CB, 3 * HG], F16)
            wo_sb = pqkv.tile([128, HPC, D], F16)

            xt_r = xt_d.rearrange("(c p) l -> p c l", p=128)
            wqkv_r = wqkv_d.rearrange("(c p) d -> p c d", p=128)
            # interleave so the group-0 Q/K matmuls can start ~3us in: per
            # chunk c, the weight chunk then the first-512 x^T columns; the
            # first weight chunk is split so the very first Q matmul's
            # operands arrive as early as possible
            nc.sync.dma_start(wqkv_sb[:, 0, 0:HG], wqkv_r[:, 0, 0:HG])
            nc.sync.dma_start(xt[:, 0, 0:512], xt_r[:, 0, 0:512])
            nc.sync.dma_start(wqkv_sb[:, 0, HG:], wqkv_r[:, 0, HG:])
            for c in range(1, CB):
                nc.sync.dma_start(wqkv_sb[:, c, :], wqkv_r[:, c, :])
                nc.sync.dma_start(xt[:, c, 0:512], xt_r[:, c, 0:512])
            for c in range(CB):
                nc.sync.dma_start(xt[:, c, 512:L_], xt_r[:, c, 512:L_])
            nc.sync.dma_start(wo_sb, wo_d.rearrange("(h p) e -> p h e", p=128))

            tri = trio[:, 0:128]
            ones = trio[:, 128:256]

            # ---- QKV group 0: chunk-major so the PE rides the arriving
            # per-chunk DMAs without stalling (3 heads' q+k accumulate in
            # 6 PSUM banks at once; attention pools are idle this early) ----
            qk_ps = {
                0: (lambda t_: (t_[:, 0, :], t_[:, 1, :]))(
                    ps_s.tile([128, 2, 512], F32, tag="ps", name="qk0")
                ),
                1: (lambda t_: (t_[:, 0, :], t_[:, 1, :]))(
                    ps_s.tile([128, 2, 512], F32, tag="ps", name="qk1")
                ),
                2: (
                    ps_o.tile([128, 512], F32, tag="po", name="qk2q"),
                    ps_sh.tile([128, 512], F32, tag="sh", name="qk2k"),
                ),
            }
            for c in range(CB):
                for h in range(HPC):
                    pq, pk = qk_ps[h]
                    nc.tensor.matmul(
                        pq, wqkv_sb[:, c, h * 128 : (h + 1) * 128],
                        xt[:, c, 0:512],
                        start=(c == 0), stop=(c == CB - 1),
                    )
                    nc.tensor.matmul(
                        pk, wqkv_sb[:, c, HG + h * 128 : HG + (h + 1) * 128],
                        xt[:, c, 0:512],
                        start=(c == 0), stop=(c == CB - 1),
                    )
            for h in range(HPC):
                pq, pk = qk_ps[h]
                nc.scalar.activation(
                    q_t[:, h, 0:512], pq, IDENT, bias=bq_sb[:, h : h + 1]
                )
                nc.scalar.activation(
                    k_t[:, h, 0:512], pk, IDENT, bias=bk_sb[:, h : h + 1]
                )
            for b in range(4):
                pv = ps_sh.tile([128, 512], F32, tag="sh", name="pv")
                for c in range(CB):
                    nc.tensor.matmul(
                        pv[:, :HG], xt[:, c, b * 128 : (b + 1) * 128],
                        wqkv_sb[:, c, 2 * HG : 3 * HG],
                        start=(c == 0), stop=(c == CB - 1),
                    )
                nc.vector.tensor_copy(v_t[:, b, :], pv[:, :HG])

            # ---- interleaved QKV(g+1) / attention(g) stream ----

            def emit_qk_unit(g, h):
                qsl = slice(g * 512, (g + 1) * 512)
                pq = ps_sh.tile([128, 512], F32, tag="sh", name="pq")
                for c in range(CB):
                    nc.tensor.matmul(
                        pq, wqkv_sb[:, c, h * 128 : (h + 1) * 128], xt[:, c, qsl],
                        start=(c == 0), stop=(c == CB - 1),
                    )
                nc.scalar.activation(
                    q_t[:, h, qsl], pq, IDENT, bias=bq_sb[:, h : h + 1]
                )
                pk = ps_sh.tile([128, 512], F32, tag="sh", name="pk")
                for c in range(CB):
                    nc.tensor.matmul(
                        pk, wqkv_sb[:, c, HG + h * 128 : HG + (h + 1) * 128],
                        xt[:, c, qsl],
                        start=(c == 0), stop=(c == CB - 1),
                    )
                nc.scalar.activation(
                    k_t[:, h, qsl], pk, IDENT, bias=bk_sb[:, h : h + 1]
                )

            def emit_v_unit(g, b):
                lb = g * 4 + b
                pv = ps_sh.tile([128, 512], F32, tag="sh", name="pv")
                for c in range(CB):
                    nc.tensor.matmul(
                        pv[:, :HG], xt[:, c, lb * 128 : (lb + 1) * 128],
                        wqkv_sb[:, c, 2 * HG : 3 * HG],
                        start=(c == 0), stop=(c == CB - 1),
                    )
                nc.vector.tensor_copy(v_t[:, lb, :], pv[:, :HG])

            # attention batches: per (g,h), j indexes pairs of 128-key blocks
            flat = []
            win_start = {}
            for g in range(NQG):
                win_start[g] = len(flat)
                nb = 2 * (g + 1)
                for h in range(HPC):
                    for pos in range(nb):
                        flat.append((g, h, pos, pos == nb - 1, pos == 0))
            state = {}
            pending = []  # (delay, closure)

            def emit_S(m):
                g, h, j, last, first = flat[m]
                ps = ps_s.tile([128, 2, 512], F32, tag="ps")
                for t in range(2):
                    kb = 2 * j + t
                    i = kb - 4 * g
                    c0 = 128 * i if i > 0 else 0
                    nc.tensor.matmul(
                        ps[:, t, c0:],
                        k_t[:, h, kb * 128 : (kb + 1) * 128],
                        q_t[:, h, g * 512 + c0 : (g + 1) * 512],
                        start=True, stop=True,
                    )
                state[m] = ps

            def emit_rest(m):
                g, h, j, last, first = flat[m]
                ps = state.pop(m)
                if first:
                    state[("po", g, h)] = ps_o.tile(
                        [128, 512], F32, tag="po", name="po"
                    )
                    state[("bs", g, h)] = pbs.tile(
                        [128, 512], F16, tag="bs", name="bsum"
                    )
                po = state[("po", g, h)]
                bsum = state[("bs", g, h)]
                at = pat.tile([128, 2, 512], F16)
                diag = j >= 2 * g
                if diag:
                    # single strided call over both key blocks, starting at
                    # the first block's causal offset; the [c0a, c0b) sliver
                    # of t=1 exp's stale PSUM that nothing ever reads
                    c0a = 128 * (2 * j - 4 * g)
                    nc.scalar.activation(
                        at[:, :, c0a:], ps[:, :, c0a:], EXP,
                        scale=SCALE, bias=eb,
                    )
                elif last:
                    # split: halves the exp latency gating this group's
                    # finalize chain
                    nc.scalar.activation(
                        at[:, 0, :], ps[:, 0, :], EXP, scale=SCALE, bias=eb
                    )
                    nc.scalar.activation(
                        at[:, 1, :], ps[:, 1, :], EXP, scale=SCALE, bias=eb
                    )
                else:
                    nc.scalar.activation(at, ps, EXP, scale=SCALE, bias=eb)
                for t in range(2):
                    kb = 2 * j + t
                    i = kb - 4 * g
                    c0 = 128 * i if i > 0 else 0
                    if i >= 0:
                        # triangle mask on the diagonal 128-block; columns
                        # left of it are never computed or read
                        nc.vector.tensor_mul(
                            at[:, t, c0 : c0 + 128], at[:, t, c0 : c0 + 128], tri
                        )
                    # Bsum accumulation on DVE (fp16 2x) replaces the
                    # ones-matmul row sums
                    if first and t == 0:
                        nc.vector.tensor_copy(bsum, at[:, 0, :])
                    else:
                        nc.vector.tensor_add(
                            bsum[:, c0:], bsum[:, c0:], at[:, t, c0:]
                        )
                    st, sp = first and t == 0, last and t == 1
                    nc.tensor.matmul(
                        po[:, c0:],
                        v_t[:, kb, h * 128 : (h + 1) * 128],
                        at[:, t, c0:],
                        start=st, stop=sp,
                    )
                if last:
                    # cross-partition reduce launches immediately; the final
                    # group head uses a PE ones-matmul (short latency, and
                    # the PE has slack in the last window) while the rest go
                    # to the otherwise-idle Pool engine
                    if g == NQG - 1 and h == HPC - 1:
                        sums_ps = ps_sh.tile(
                            [128, 512], F32, tag="sh", name="sums_ps"
                        )
                        nc.tensor.matmul(
                            sums_ps, ones, bsum, start=True, stop=True
                        )
                        sums = pn.tile([128, 512], F32, tag="sums")
                        nc.vector.tensor_copy(sums, sums_ps)
                    else:
                        sums = pn.tile([128, 512], F32, tag="sums")
                        nc.gpsimd.partition_all_reduce(
                            sums, bsum, 128, bass_isa.ReduceOp.add
                        )
                    state[("sm", g, h)] = sums

            def emit_finalize(g, h):
                def run():
                    po = state.pop(("po", g, h))
                    state.pop(("bs", g, h))
                    sums = state.pop(("sm", g, h))
                    recip = pn.tile([128, 512], F32, tag="recip")
                    nc.vector.reciprocal(recip, sums)
                    nc.vector.tensor_mul(
                        o_t[:, h, g * 512 : (g + 1) * 512], po, recip
                    )
                return run

            def emit_proj_lb(g, b):
                def run():
                    lb = g * 4 + b
                    lsl = slice(lb * 128, (lb + 1) * 128)
                    final = g == NQG - 1 and b == 3
                    ysb = py_.tile([128, D], F16, tag="ysb")
                    for eh in range(2):
                        pyp = ps_sh.tile([128, 512], F32, tag="sh", name="pyp")
                        for h in range(HPC):
                            nc.tensor.matmul(
                                pyp[:, :384],
                                o_t[:, h, lsl],
                                wo_sb[:, h, eh * 384 : (eh + 1) * 384],
                                start=(h == 0), stop=(h == HPC - 1),
                            )
                        if eh == 0:
                            nc.vector.tensor_copy(ysb[:, 0:384], pyp[:, :384])
                        else:
                            nc.scalar.activation(
                                ysb[:, 384:768], pyp[:, :384], IDENT, bias=0.0
                            )
                        if final:
                            # split the very last store so its first half
                            # overlaps the second half's matmuls
                            nc.sync.dma_start(
                                y_d[lsl, eh * 384 : (eh + 1) * 384],
                                ysb[:, eh * 384 : (eh + 1) * 384],
                            )
                    if not final:
                        nc.sync.dma_start(y_d[lsl, :], ysb)
                return run

            # per-window QKV filler units: window g interleaves group g+1's
            # QKV work between group g's attention batches
            def window_units(g):
                if g + 1 >= NQG:
                    return []
                u = [("qk", g + 1, h) for h in range(HPC)]
                u += [("v", g + 1, b) for b in range(4)]
                return u

            def emit_unit(u):
                kind, g, i = u
                if kind == "qk":
                    emit_qk_unit(g, i)
                else:
                    emit_v_unit(g, i)

            emit_S(0)
            for g in range(NQG):
                units = window_units(g)
                nbatch = 6 * (g + 1)
                emitted_u = 0
                for bi in range(nbatch):
                    m = win_start[g] + bi
                    if m + 1 < len(flat):
                        emit_S(m + 1)
                    # spread this window's QKV units across its batches
                    want = (len(units) * (bi + 1)) // nbatch
                    while emitted_u < want:
                        emit_unit(units[emitted_u])
                        emitted_u += 1
                    nxt = []
                    for d, fn in pending:
                        if d <= 0:
                            fn()
                        else:
                            nxt.append((d - 1, fn))
                    pending = nxt
                    emit_rest(m)
                    _, h, j, last, first = flat[m]
                    if last:
                        pending.append((1, emit_finalize(g, h)))
                        if h == HPC - 1:
                            # spread the projection's L-blocks so the PSUM
                            # slot rotation hides each pyp's copy latency
                            for b in range(4):
                                pending.append((2 + b, emit_proj_lb(g, b)))
            for d, fn in sorted(pending, key=lambda p: p[0]):
                fn()

    nc.compile()
    return nc


_NC_CACHE = {}


def _get_nc(L_=L):
    if L_ not in _NC_CACHE:
        _NC_CACHE[L_] = build_nc(L_)
    return _NC_CACHE[L_]


def run_sharded(inputs, L_=L, trace=False):
    """Shard inputs over 8 cores, run, return results object."""
    x = np.asarray(inputs["x_input"], dtype=np.float32)
    tri = (np.arange(128)[None, :] >= np.arange(128)[:, None]).astype(np.float16)
    trio = np.concatenate([tri, np.ones((128, 128), np.float16)], axis=1)
    eb = np.full((128, 1), EXP_BIAS, dtype=np.float32)
    in_maps = []
    for c in range(N_CORES):
        b, gslice = c // 2, slice((c % 2) * HG, (c % 2) * HG + HG)
        wqkv = np.concatenate(
            [
                np.asarray(inputs["Wq"], np.float32)[:, gslice],
                np.asarray(inputs["Wk"], np.float32)[:, gslice],
                np.asarray(inputs["Wv"], np.float32)[:, gslice],
            ],
            axis=1,
        ).astype(np.float16)
        in_maps.append(
            {
                "xt": np.ascontiguousarray(x[b].T.astype(np.float16)),
                "wqkv": np.ascontiguousarray(wqkv),
                "wo": np.ascontiguousarray(
                    np.asarray(inputs["Wo"], np.float32)[gslice, :].astype(np.float16)
                ),
                "bq": np.ascontiguousarray(
                    np.asarray(inputs["bq"], np.float32)[gslice]
                ),
                "bk": np.ascontiguousarray(
                    np.asarray(inputs["bk"], np.float32)[gslice]
                ),
                "tri": trio,
                "eb": eb,
            }
        )
    nc = _get_nc(L_)
    try:
        res = run_bass_kernel_spmd(nc, in_maps, list(range(N_CORES)), trace=trace)
    except Exception:
        # transient device faults (NRT_EXEC_UNIT_UNRECOVERABLE etc.): one retry
        res = run_bass_kernel_spmd(nc, in_maps, list(range(N_CORES)), trace=trace)
    return res


def kernel(**inputs) -> np.ndarray:
    res = run_sharded(inputs)
    # host-side unshard: sum the two head-group partials per batch; add the
    # bias terms that commute out of the device computation exactly:
    # softmax rows sum to 1, so  A @ (xWv + bv) Wo + bo = A(xWv)Wo + bv@Wo + bo
    bias = (
        np.asarray(inputs["bv"], np.float32) @ np.asarray(inputs["Wo"], np.float32)
        + np.asarray(inputs["bo"], np.float32)
    )
    out = np.empty((B, L, D), dtype=np.float32)
    for b in range(B):
        out[b] = (
            res.results[2 * b]["y"].astype(np.float32)
            + res.results[2 * b + 1]["y"].astype(np.float32)
            + bias
        )
    return out
